# revision 9
# baseline (speedup 1.0000x reference)
"""Trainium2 Bass kernel: batched 4-point DLT homography (closed-form solve).

Contract: kernel(pts_1_tile, pred_h4p_tile) -> [B, 3, 3] float32, with
B = 524288 split across 8 NeuronCores (batch-parallel, no communication).

Math (per batch element, points p=0..3 with src (x_p,y_p), dst (X_p,Y_p)):
the DLT system rows are
    x h0 + y h1 + h2 = X (1 + x h6 + y h7)
    x h3 + y h4 + h5 = Y (1 + x h6 + y h7)
Eliminating (h0,h1,h2) from the four X-equations via the left null vector n
of M = [(x_p, y_p, 1)] gives one linear equation in (h6,h7); same for the
Y-equations. Solve the 2x2, back out the rest in closed form. Matches
jnp.linalg.solve on the 8x8 to ~1e-6 absmax (well-conditioned inputs).

Layout: each core's 65536 elements sit at [128 partitions, F free]; every
scalar of the formula is a [128, F] plane and every step is an elementwise
plane op, split across VectorE / GPSIMD (ScalarE does the two reciprocals).
Input comp k of element f lives at free index f*8+k of the raw input tiles
(strided AP views; no deinterleave pass), output h_j written straight into
the interleaved [128, F*9] output tile.
"""
import sys

for _p in ("/opt/trn_rl_repo", "/root/.axon_site/_ro/trn_rl_repo"):
    if _p not in sys.path:
        sys.path.append(_p)

import numpy as np

import concourse.bass as bass
import concourse.mybir as mybir
from concourse import bacc
from concourse.tile import TileContext
from concourse.bass_utils import run_bass_kernel_spmd

N_CORES = 8
B_TOTAL = 524288
PER_CORE = B_TOTAL // N_CORES  # 65536
PARTS = 128
F = PER_CORE // PARTS  # 512
FP32 = mybir.dt.float32

ADD = mybir.AluOpType.add
SUB = mybir.AluOpType.subtract
MUL = mybir.AluOpType.mult


class _Ctx:
    """Plane allocator (free-list, bounds SBUF to max-live planes) plus a
    greedy static DVE/GPSIMD balance by estimated op time."""

    def __init__(self, nc, pool, fc):
        self.nc = nc
        self.pool = pool
        self.fc = fc
        self.planes = {}
        self.free = []
        self.n_tags = 0
        self.t_v = 0.0
        self.t_g = 0.0
        self.cost_v = (fc + 151.0) / 0.96
        self.cost_g = fc * 2.6 / 1.2 + 300.0

    def P(self, name):
        if name not in self.planes:
            if self.free:
                t = self.free.pop()
            else:
                t = self.pool.tile([PARTS, self.fc], FP32, tag=f"pl{self.n_tags}")
                self.n_tags += 1
            self.planes[name] = t
        return self.planes[name]

    def kill(self, *names):
        for nm in names:
            self.free.append(self.planes.pop(nm))

    def tt(self, out, a, b, op, pin=None):
        eng = pin
        if eng is None:
            eng = "v" if self.t_v + self.cost_v <= self.t_g + self.cost_g else "g"
        if eng == "v":
            self.nc.vector.tensor_tensor(out=out, in0=a, in1=b, op=op)
            self.t_v += self.cost_v
        else:
            self.nc.gpsimd.tensor_tensor(out=out, in0=a, in1=b, op=op)
            self.t_g += self.cost_g

    def stt(self, out, in0, scalar, in1, op0, op1):
        # (in0 op0 scalar) op1 in1 ; DVE only
        self.nc.vector.scalar_tensor_tensor(
            out=out, in0=in0, scalar=scalar, in1=in1, op0=op0, op1=op1
        )
        self.t_v += self.cost_v


def _chunk_body(nc, cx, vt, pt, ut, o9, v8, u8):
    P, tt, stt, kill = cx.P, cx.tt, cx.stt, cx.kill
    x = [v8[:, 2 * p, :] for p in range(4)]
    y = [v8[:, 2 * p + 1, :] for p in range(4)]
    X = [u8[:, 2 * p, :] for p in range(4)]
    Y = [u8[:, 2 * p + 1, :] for p in range(4)]

    def recip(dst, src):
        # ~2 ULP, two DVE ops
        nc.vector.reciprocal_approx_accurate(out=dst, in_=src, scratch=P("rc_s"))
        kill("rc_s")
        cx.t_v += 2 * cx.cost_v

    half = cx.fc * 4
    tt(ut[:, :half], vt[:, :half], pt[:, :half], ADD, pin="v")
    tt(ut[:, half:], vt[:, half:], pt[:, half:], ADD, pin="g")

    nc.vector.memset(o9[:, 8, :], 1.0)

    # diffs (strided reads -> DVE)
    for i in (1, 2, 3):
        tt(P(f"dx{i}"), x[i], x[0], SUB, pin="v")
        tt(P(f"dy{i}"), y[i], y[0], SUB, pin="v")

    # null vector n of [(x_p, y_p, 1)]^T ; n0 = -s
    for nm, (a1, b1, a2, b2) in {
        "n1": ("dx2", "dy3", "dx3", "dy2"),
        "n2": ("dx3", "dy1", "dx1", "dy3"),
        "n3": ("dx1", "dy2", "dx2", "dy1"),
    }.items():
        tt(P(nm + "_a"), P(a1), P(b1), MUL)
        tt(P(nm + "_b"), P(a2), P(b2), MUL)
        tt(P(nm), P(nm + "_a"), P(nm + "_b"), SUB)
        kill(nm + "_a", nm + "_b")
    kill("dx3", "dy3")
    tt(P("s_1"), P("n1"), P("n2"), ADD)
    tt(P("s"), P("s_1"), P("n3"), ADD)
    kill("s_1")

    # a = sum n_p W_p, b = sum n_p W_p x_p, c = sum n_p W_p y_p for W in (X, Y)
    for side, W in (("X", X), ("Y", Y)):
        for i in (1, 2, 3):
            tt(P(f"z{i}"), P(f"n{i}"), W[i], MUL, pin="v")
        tt(P("z0"), P("s"), W[0], MUL, pin="v")
        for nm, lane in ((f"a{side}", None), (f"b{side}", x), (f"c{side}", y)):
            if lane is None:
                q = [P(f"z{i}") for i in (1, 2, 3, 0)]
            else:
                q = []
                for i in (1, 2, 3, 0):
                    tt(P(f"q{i}"), P(f"z{i}"), lane[i], MUL, pin="v")
                    q.append(P(f"q{i}"))
            tt(P(nm + "_1"), q[0], q[1], ADD)
            tt(P(nm + "_2"), P(nm + "_1"), q[2], ADD)
            tt(P(nm), P(nm + "_2"), q[3], SUB)
            kill(nm + "_1", nm + "_2")
            if lane is not None:
                kill("q1", "q2", "q3", "q0")
        kill("z1", "z2", "z3", "z0")
    kill("n1", "n2", "s")

    # 2x2 solve for (h6, h7); h6/h7 written straight to output lanes
    tt(P("dt_a"), P("bX"), P("cY"), MUL)
    tt(P("dt_b"), P("bY"), P("cX"), MUL)
    tt(P("det"), P("dt_a"), P("dt_b"), SUB)
    recip(P("rdet"), P("det"))
    kill("dt_a", "dt_b", "det")
    h6, h7 = o9[:, 6, :], o9[:, 7, :]
    tt(P("h6_a"), P("cX"), P("aY"), MUL)
    tt(P("h6_b"), P("cY"), P("aX"), MUL)
    tt(P("h6_n"), P("h6_a"), P("h6_b"), SUB)
    tt(h6, P("h6_n"), P("rdet"), MUL, pin="v")
    tt(P("h7_a"), P("bY"), P("aX"), MUL)
    tt(P("h7_b"), P("bX"), P("aY"), MUL)
    tt(P("h7_n"), P("h7_a"), P("h7_b"), SUB)
    tt(h7, P("h7_n"), P("rdet"), MUL, pin="v")
    kill("h6_a", "h6_b", "h6_n", "h7_a", "h7_b", "h7_n", "rdet")
    kill("aX", "bX", "cX", "aY", "bY", "cY")

    # XW_p = X_p (1 + x_p h6 + y_p h7), YW_p likewise; only p=0,1,2 needed
    for p in range(3):
        tt(P(f"m1_{p}"), x[p], h6, MUL, pin="v")
        tt(P(f"m2_{p}"), y[p], h7, MUL, pin="v")
        tt(P(f"sp{p}"), P(f"m1_{p}"), P(f"m2_{p}"), ADD)
        stt(P(f"XW{p}"), P(f"sp{p}"), 1.0, X[p], ADD, MUL)
        stt(P(f"YW{p}"), P(f"sp{p}"), 1.0, Y[p], ADD, MUL)
        kill(f"m1_{p}", f"m2_{p}", f"sp{p}")

    for nm, a, b in (
        ("P1", "XW1", "XW0"),
        ("P2", "XW2", "XW0"),
        ("Q1", "YW1", "YW0"),
        ("Q2", "YW2", "YW0"),
    ):
        tt(P(nm), P(a), P(b), SUB)
    kill("XW1", "XW2", "YW1", "YW2")

    recip(P("rD"), P("n3"))
    kill("n3")

    # back-substitution: (h0,h1,h2) from P*, (h3,h4,h5) from Q*
    for (r1, r2, wp, o0, o1, o2) in (
        ("P1", "P2", "XW0", 0, 1, 2),
        ("Q1", "Q2", "YW0", 3, 4, 5),
    ):
        ho0, ho1, ho2 = o9[:, o0, :], o9[:, o1, :], o9[:, o2, :]
        tt(P("g_a"), P(r1), P("dy2"), MUL)
        tt(P("g_b"), P(r2), P("dy1"), MUL)
        tt(P("g_n"), P("g_a"), P("g_b"), SUB)
        tt(ho0, P("g_n"), P("rD"), MUL, pin="v")
        tt(P("g_c"), P("dx1"), P(r2), MUL)
        tt(P("g_d"), P("dx2"), P(r1), MUL)
        tt(P("g_m"), P("g_c"), P("g_d"), SUB)
        tt(ho1, P("g_m"), P("rD"), MUL, pin="v")
        tt(P("g_e"), x[0], ho0, MUL, pin="v")
        tt(P("g_f"), P(wp), P("g_e"), SUB, pin="v")
        tt(P("g_g"), y[0], ho1, MUL, pin="v")
        tt(ho2, P("g_f"), P("g_g"), SUB, pin="v")
        kill("g_a", "g_b", "g_n", "g_c", "g_d", "g_m", "g_e", "g_f", "g_g")
        kill(r1, r2, wp)
    kill("rD", "dx1", "dx2", "dy1", "dy2")


def _build(nchunk=1):
    fc = F // nchunk
    elems = PARTS * fc

    nc = bacc.Bacc(None, target_bir_lowering=False, debug=True)
    pts = nc.dram_tensor("pts", [PER_CORE, 8], FP32, kind="ExternalInput")
    prd = nc.dram_tensor("prd", [PER_CORE, 8], FP32, kind="ExternalInput")
    out = nc.dram_tensor("out", [PER_CORE, 9], FP32, kind="ExternalOutput")

    with TileContext(nc) as tc:
        nb = 1 if nchunk == 1 else 2
        with tc.tile_pool(name="io", bufs=nb) as io_pool, tc.tile_pool(
            name="scratch", bufs=nb
        ) as sp:
            for c in range(nchunk):
                vt = io_pool.tile([PARTS, fc * 8], FP32, tag="vt")
                pt = io_pool.tile([PARTS, fc * 8], FP32, tag="pt")
                ot = io_pool.tile([PARTS, fc * 9], FP32, tag="ot")
                ut = sp.tile([PARTS, fc * 8], FP32, tag="ut")
                lo = c * elems
                hi = lo + elems
                nc.sync.dma_start(
                    out=vt[:, :],
                    in_=pts[lo:hi, :].rearrange("(p f) c -> p (f c)", p=PARTS),
                )
                nc.sync.dma_start(
                    out=pt[:, :],
                    in_=prd[lo:hi, :].rearrange("(p f) c -> p (f c)", p=PARTS),
                )
                cx = _Ctx(nc, sp, fc)
                v8 = vt.rearrange("p (f c) -> p c f", c=8)
                u8 = ut.rearrange("p (f c) -> p c f", c=8)
                o9 = ot.rearrange("p (f c) -> p c f", c=9)
                _chunk_body(nc, cx, vt, pt, ut, o9, v8, u8)
                nc.sync.dma_start(
                    out=out[lo:hi, :].rearrange("(p f) c -> p (f c)", p=PARTS),
                    in_=ot[:, :],
                )
    nc.finalize()
    return nc


_NC_CACHE = {}


def _get_nc(nchunk=1):
    if nchunk not in _NC_CACHE:
        _NC_CACHE[nchunk] = _build(nchunk)
    return _NC_CACHE[nchunk]


def kernel(pts_1_tile, pred_h4p_tile, _trace=False, _nchunk=1):
    pts = np.ascontiguousarray(
        np.asarray(pts_1_tile, dtype=np.float32).reshape(B_TOTAL, 8)
    )
    prd = np.ascontiguousarray(
        np.asarray(pred_h4p_tile, dtype=np.float32).reshape(B_TOTAL, 8)
    )
    nc = _get_nc(_nchunk)
    in_maps = [
        {
            "pts": pts[i * PER_CORE : (i + 1) * PER_CORE],
            "prd": prd[i * PER_CORE : (i + 1) * PER_CORE],
        }
        for i in range(N_CORES)
    ]
    res = run_bass_kernel_spmd(nc, in_maps, list(range(N_CORES)), trace=_trace)
    outs = np.concatenate([res.results[i]["out"] for i in range(N_CORES)], axis=0)
    H = outs.reshape(B_TOTAL, 3, 3).astype(np.float32)
    if _trace:
        return H, res
    return H


# revision 14
# speedup vs baseline: 1.1970x; 1.1970x over previous
"""Trainium2 Bass kernel: batched 4-point DLT homography (closed-form solve).

Contract: kernel(pts_1_tile, pred_h4p_tile) -> [B, 3, 3] float32, with
B = 524288 split across 8 NeuronCores (batch-parallel, no communication).

Math (per batch element, points p=0..3 with src (x_p,y_p), dst (X_p,Y_p)):
the DLT system rows are
    x h0 + y h1 + h2 = X (1 + x h6 + y h7)
    x h3 + y h4 + h5 = Y (1 + x h6 + y h7)
Eliminating (h0,h1,h2) from the four X-equations via the left null vector n
of M = [(x_p, y_p, 1)] gives one linear equation in (h6,h7); same for the
Y-equations. Solve the 2x2, back out the rest in closed form. Matches
jnp.linalg.solve on the 8x8 to ~1e-6 absmax (well-conditioned inputs).

Layout: each core's 65536 elements sit at [128 partitions, 512 free]; every
per-element scalar is a [128, 512] "plane". Planes live at fixed offsets in
one SBUF slab so related planes are contiguous and most steps fuse into
multi-plane single instructions (all positive-step / broadcast APs — DVE
runs those at full 1x rate). ScalarE does the interleave<->planar shuffles,
VectorE + GPSIMD split the elementwise math, greedy-balanced.
"""
import sys

for _p in ("/opt/trn_rl_repo", "/root/.axon_site/_ro/trn_rl_repo"):
    if _p not in sys.path:
        sys.path.append(_p)

import numpy as np

import concourse.bass as bass
import concourse.mybir as mybir
from concourse import bacc
from concourse.tile import TileContext
from concourse.bass_utils import run_bass_kernel_spmd

N_CORES = 8
B_TOTAL = 524288
PER_CORE = B_TOTAL // N_CORES  # 65536
PARTS = 128
F = PER_CORE // PARTS  # 512
FP32 = mybir.dt.float32

ADD = mybir.AluOpType.add
SUB = mybir.AluOpType.subtract
MUL = mybir.AluOpType.mult

NPLANES = 94  # slab planes per partition (94*2KB = 188KB of 192KB budget)


class _Slab:
    """Bump allocator with explicit free over one big SBUF tile, in units of
    F-sized planes. Returns (offset, length) regions; first-fit reuse."""

    def __init__(self, nplanes):
        self.free = [(0, nplanes)]
        self.peak = 0
        self.nplanes = nplanes

    def alloc(self, n):
        for idx, (off, ln) in enumerate(self.free):
            if ln >= n:
                if ln == n:
                    self.free.pop(idx)
                else:
                    self.free[idx] = (off + n, ln - n)
                self.peak = max(self.peak, off + n)
                return off
        raise RuntimeError(f"slab OOM: need {n}, free={self.free}")

    def release(self, off, n):
        self.free.append((off, n))
        self.free.sort()
        merged = []
        for o, ln in self.free:
            if merged and merged[-1][0] + merged[-1][1] == o:
                merged[-1] = (merged[-1][0], merged[-1][1] + ln)
            else:
                merged.append((o, ln))
        self.free = [tuple(m) for m in merged]


class _Bal:
    """Greedy VectorE/GPSIMD balance by estimated op time (ns)."""

    def __init__(self, nc):
        self.nc = nc
        self.t_v = 0.0
        self.t_g = 0.0

    def pick(self, fd, pin):
        cv = (fd + 151.0) / 0.96
        cg = fd * 4.0 / 1.2 + 300.0
        if pin is None:
            eng = "v" if self.t_v + cv <= self.t_g + cg else "g"
        else:
            eng = pin
        if eng == "v":
            self.t_v += cv
            return self.nc.vector
        self.t_g += cg
        return self.nc.gpsimd


def _fd(ap):
    n = 1
    for d in ap.shape[1:]:
        n *= d
    return n


def _build(nchunk=1):
    fc = F // nchunk
    elems = PARTS * fc

    nc = bacc.Bacc(None, target_bir_lowering=False, debug=True)
    pts = nc.dram_tensor("pts", [PER_CORE, 8], FP32, kind="ExternalInput")
    prd = nc.dram_tensor("prd", [PER_CORE, 8], FP32, kind="ExternalInput")
    out = nc.dram_tensor("out", [PER_CORE, 9], FP32, kind="ExternalOutput")

    with TileContext(nc) as tc:
        with tc.tile_pool(name="slab", bufs=1) as pool:
            slab = pool.tile([PARTS, NPLANES * fc], FP32, tag="slab")

            def R(off, n):  # flat region view [P, n*fc]
                return slab[:, off * fc : (off + n) * fc]

            def V(off, n):  # per-plane view [P, n, fc]
                return R(off, n).rearrange("p (c f) -> p c f", f=fc)

            def PL(off):  # one plane [P, fc]
                return slab[:, off * fc : (off + 1) * fc]

            def BC(off, k):  # one plane broadcast k times -> [P, k, fc]
                return PL(off).unsqueeze(1).broadcast_to((PARTS, k, fc))

            for c in range(nchunk):
                sa = _Slab(NPLANES)
                bal = _Bal(nc)

                def tt(o, a, b, op, pin=None):
                    eng = bal.pick(_fd(o), pin)
                    eng.tensor_tensor(out=o, in0=a, in1=b, op=op)

                def stt(o, in0, scalar, in1, op0, op1):
                    bal.t_v += (_fd(o) + 151.0) / 0.96
                    nc.vector.scalar_tensor_tensor(
                        out=o, in0=in0, scalar=scalar, in1=in1, op0=op0, op1=op1
                    )

                def scp(o, i):  # ScalarE copy (shuffles)
                    nc.scalar.copy(out=o, in_=i)

                lo = c * elems
                hi = lo + elems

                vt = sa.alloc(8)
                pt = sa.alloc(8)
                ut = sa.alloc(8)
                nc.sync.dma_start(
                    out=R(vt, 8),
                    in_=pts[lo:hi, :].rearrange("(p f) c -> p (f c)", p=PARTS),
                )
                nc.sync.dma_start(
                    out=R(pt, 8),
                    in_=prd[lo:hi, :].rearrange("(p f) c -> p (f c)", p=PARTS),
                )

                # interleaved u = v + pred (contiguous; split 6F/2F across V/G)
                tt(R(ut, 8)[:, : 6 * fc], R(vt, 8)[:, : 6 * fc],
                   R(pt, 8)[:, : 6 * fc], ADD, pin="v")
                tt(R(ut, 8)[:, 6 * fc :], R(vt, 8)[:, 6 * fc :],
                   R(pt, 8)[:, 6 * fc :], ADD, pin="g")

                # deinterleave: comp order (0,2,4,6,1,3,5,7) -> [x0..x3,y0..y3]
                xv = sa.alloc(8)  # [x0,x1,x2,x3,y0,y1,y2,y3]
                uu = sa.alloc(8)  # [X0,X1,X2,X3,Y0,Y1,Y2,Y3]

                def deint(dst, src):
                    i = R(src, 8).rearrange("p (f c g) -> p g c f", c=4, g=2)
                    o = R(dst, 8).rearrange("p (g c f) -> p g c f", c=4, g=2)
                    scp(o, i)

                deint(xv, vt)
                deint(uu, ut)
                sa.release(pt, 8)
                sa.release(ut, 8)
                sa.release(vt, 8)
                ot = sa.alloc(9)
                # OT is element-interleaved (f*9 + c) so the out-DMA is contiguous
                ov = R(ot, 9).rearrange("p (f c) -> p c f", c=9)

                nc.vector.memset(ov[:, 8, :], 1.0)

                # diffs: D = [dx1,dx2,dx3,dy1,dy2,dy3]
                dd = sa.alloc(6)
                xv4 = R(xv, 8).rearrange("p (g c f) -> p g c f", g=2, c=4)
                in0 = xv4[:, :, 1:4, :]
                in1 = (
                    V(xv, 8)[:, 0:5:4, :]
                    .unsqueeze(2)
                    .broadcast_to((PARTS, 2, 3, fc))
                )
                o = R(dd, 6).rearrange("p (g c f) -> p g c f", g=2, c=3)
                tt(o, in0, in1, SUB, pin="v")
                DX1, DX2, DX3, DY1, DY2, DY3 = range(dd, dd + 6)

                # n vector: n1=dx2dy3-dx3dy2, n2=dx3dy1-dx1dy3, n3=dx1dy2-dx2dy1
                pa = sa.alloc(3)
                pb = sa.alloc(3)
                ns = sa.alloc(4)  # [n0, n1, n2, n3]
                for k, (a, b) in enumerate(((DX2, DY3), (DX3, DY1), (DX1, DY2))):
                    tt(PL(pa + k), PL(a), PL(b), MUL)
                for k, (a, b) in enumerate(((DX3, DY2), (DX1, DY3), (DX2, DY1))):
                    tt(PL(pb + k), PL(a), PL(b), MUL)
                tt(R(ns + 1, 3), R(pa, 3), R(pb, 3), SUB)
                t0 = sa.alloc(1)
                tt(PL(t0), PL(ns + 1), PL(ns + 2), ADD)
                # n0 = -(n1+n2) - n3
                stt(PL(ns), PL(t0), -1.0, PL(ns + 3), MUL, SUB)
                sa.release(pa, 3)
                sa.release(pb, 3)
                sa.release(t0, 1)

                # dots: ZW[p] = (z_p, z_p x_p, z_p y_p) grouped by p, both sides
                zx = sa.alloc(12)
                zy = sa.alloc(12)
                for zz, w in ((zx, 0), (zy, 4)):
                    tt(V(zz, 12)[:, 0:12:3, :], V(ns, 4), V(uu, 8)[:, w : w + 4, :],
                       MUL)
                    tt(V(zz, 12)[:, 1:12:3, :], V(zz, 12)[:, 0:12:3, :],
                       V(xv, 8)[:, 0:4, :], MUL)
                    tt(V(zz, 12)[:, 2:12:3, :], V(zz, 12)[:, 0:12:3, :],
                       V(xv, 8)[:, 4:8, :], MUL)
                tx = sa.alloc(6)
                ty = sa.alloc(6)
                ss = sa.alloc(6)  # [aX,bX,cX,aY,bY,cY]
                tt(R(tx, 6), R(zx, 6), R(zx + 6, 6), ADD)
                tt(R(ty, 6), R(zy, 6), R(zy + 6, 6), ADD)
                tt(R(ss, 3), R(tx, 3), R(tx + 3, 3), ADD)
                tt(R(ss + 3, 3), R(ty, 3), R(ty + 3, 3), ADD)
                sa.release(zx, 12)
                sa.release(zy, 12)
                sa.release(tx, 6)
                sa.release(ty, 6)

                # 2x2 solve: det = bX cY - bY cX ; h6n = cX aY - cY aX ;
                # h7n = bY aX - bX aY
                AX, BX, CX, AY, BY, CY = range(ss, ss + 6)
                pc = sa.alloc(3)
                pd = sa.alloc(3)
                dt = sa.alloc(3)  # [det, h6n, h7n]
                for k, (a, b) in enumerate(((BX, CY), (CX, AY), (BY, AX))):
                    tt(PL(pc + k), PL(a), PL(b), MUL)
                for k, (a, b) in enumerate(((BY, CX), (CY, AX), (BX, AY))):
                    tt(PL(pd + k), PL(a), PL(b), MUL)
                tt(R(dt, 3), R(pc, 3), R(pd, 3), SUB)
                sa.release(pc, 3)
                sa.release(pd, 3)

                rc = sa.alloc(2)  # [recip, scratch]
                rdet = sa.alloc(1)
                nc.vector.reciprocal_approx_accurate(
                    out=PL(rdet), in_=PL(dt), scratch=PL(rc)
                )
                bal.t_v += 2 * (fc + 151.0) / 0.96
                h67 = sa.alloc(2)
                tt(V(h67, 2), V(dt + 1, 2), BC(rdet, 2), MUL, pin="v")
                scp(ov[:, 6:8, :], V(h67, 2))
                sa.release(dt, 3)
                sa.release(rdet, 1)
                sa.release(ss, 6)

                # XW_p = X_p (1 + x_p h6 + y_p h7), p=0..2; same for YW
                m1 = sa.alloc(3)
                m2 = sa.alloc(3)
                sp = sa.alloc(3)
                xw = sa.alloc(6)  # [XW0,XW1,XW2,YW0,YW1,YW2]
                tt(V(m1, 3), V(xv, 8)[:, 0:3, :], BC(h67, 3), MUL, pin="v")
                tt(V(m2, 3), V(xv, 8)[:, 4:7, :], BC(h67 + 1, 3), MUL, pin="v")
                tt(R(sp, 3), R(m1, 3), R(m2, 3), ADD)
                stt(V(xw, 6)[:, 0:3, :], V(sp, 3), 1.0, V(uu, 8)[:, 0:3, :],
                    ADD, MUL)
                stt(V(xw, 6)[:, 3:6, :], V(sp, 3), 1.0, V(uu, 8)[:, 4:7, :],
                    ADD, MUL)
                sa.release(m1, 3)
                sa.release(m2, 3)
                sa.release(sp, 3)
                sa.release(h67, 2)
                sa.release(uu, 8)

                # PQ = (XW1-XW0, XW2-XW0, YW1-YW0, YW2-YW0)
                pq = sa.alloc(4)
                xwv = R(xw, 6).rearrange("p (a b f) -> p a b f", a=2, b=3)
                tt(
                    R(pq, 4).rearrange("p (a b f) -> p a b f", a=2, b=2),
                    xwv[:, :, 1:3, :],
                    xwv[:, :, 0, :].unsqueeze(2).broadcast_to((PARTS, 2, 2, fc)),
                    SUB,
                    pin="v",
                )
                P1, P2, Q1, Q2 = range(pq, pq + 4)

                rd_ = sa.alloc(1)
                nc.vector.reciprocal_approx_accurate(
                    out=PL(rd_), in_=PL(ns + 3), scratch=PL(rc)
                )
                bal.t_v += 2 * (fc + 151.0) / 0.96
                sa.release(ns, 4)

                # pE = (P1 dy2, Q1 dy2, dx1 P2, dx1 Q2)
                # pF = (P2 dy1, Q2 dy1, dx2 P1, dx2 Q1)
                pe = sa.alloc(4)
                pf = sa.alloc(4)
                pqv = V(pq, 4)
                tt(V(pe, 4)[:, 0:2, :], pqv[:, 0:3:2, :], BC(DY2, 2), MUL, pin="v")
                tt(V(pe, 4)[:, 2:4, :], pqv[:, 1:4:2, :], BC(DX1, 2), MUL, pin="v")
                tt(V(pf, 4)[:, 0:2, :], pqv[:, 1:4:2, :], BC(DY1, 2), MUL, pin="v")
                tt(V(pf, 4)[:, 2:4, :], pqv[:, 0:3:2, :], BC(DX2, 2), MUL, pin="v")
                hn = sa.alloc(4)  # [h0n, h3n, h1n, h4n]
                tt(R(hn, 4), R(pe, 4), R(pf, 4), SUB)
                hg = sa.alloc(4)  # [h0, h3, h1, h4]
                tt(V(hg, 4), V(hn, 4), BC(rd_, 4), MUL, pin="v")
                sa.release(pe, 4)
                sa.release(pf, 4)
                sa.release(hn, 4)
                sa.release(pq, 4)
                sa.release(rd_, 1)
                sa.release(rc, 2)

                scp(ov[:, 0:4:3, :], V(hg, 2))       # h0, h3
                scp(ov[:, 1:5:3, :], V(hg + 2, 2))   # h1, h4

                # h2 = XW0 - x0 h0 - y0 h1 ; h5 = YW0 - x0 h3 - y0 h4
                ee = sa.alloc(4)  # (x0 h0, y0 h1, x0 h3, y0 h4)
                in0 = (
                    V(xv, 8)[:, 0:5:4, :]
                    .unsqueeze(1)
                    .broadcast_to((PARTS, 2, 2, fc))
                )
                in1 = R(hg, 4).rearrange("p (a b f) -> p b a f", a=2, b=2)
                tt(R(ee, 4).rearrange("p (g h f) -> p g h f", g=2, h=2),
                   in0, in1, MUL, pin="v")
                s1 = sa.alloc(2)
                eev = V(ee, 4)
                tt(V(s1, 2), V(xw, 6)[:, 0:4:3, :], eev[:, 0:3:2, :], SUB, pin="v")
                h25 = sa.alloc(2)
                tt(V(h25, 2), V(s1, 2), eev[:, 1:4:2, :], SUB, pin="v")
                scp(ov[:, 2:6:3, :], V(h25, 2))
                sa.release(ee, 4)
                sa.release(s1, 2)
                sa.release(hg, 4)
                sa.release(xw, 6)
                sa.release(dd, 6)
                sa.release(xv, 8)

                nc.sync.dma_start(
                    out=out[lo:hi, :].rearrange("(p f) c -> p (f c)", p=PARTS),
                    in_=R(ot, 9),
                )
                sa.release(h25, 2)
                sa.release(ot, 9)
    nc.finalize()
    return nc


_NC_CACHE = {}


def _get_nc(nchunk=1):
    if nchunk not in _NC_CACHE:
        _NC_CACHE[nchunk] = _build(nchunk)
    return _NC_CACHE[nchunk]


def kernel(pts_1_tile, pred_h4p_tile, _trace=False, _nchunk=1):
    pts = np.ascontiguousarray(
        np.asarray(pts_1_tile, dtype=np.float32).reshape(B_TOTAL, 8)
    )
    prd = np.ascontiguousarray(
        np.asarray(pred_h4p_tile, dtype=np.float32).reshape(B_TOTAL, 8)
    )
    nc = _get_nc(_nchunk)
    in_maps = [
        {
            "pts": pts[i * PER_CORE : (i + 1) * PER_CORE],
            "prd": prd[i * PER_CORE : (i + 1) * PER_CORE],
        }
        for i in range(N_CORES)
    ]
    res = run_bass_kernel_spmd(nc, in_maps, list(range(N_CORES)), trace=_trace)
    outs = np.concatenate([res.results[i]["out"] for i in range(N_CORES)], axis=0)
    H = outs.reshape(B_TOTAL, 3, 3).astype(np.float32)
    if _trace:
        return H, res
    return H


# revision 18
# speedup vs baseline: 1.5501x; 1.2950x over previous
"""Trainium2 Bass kernel: batched 4-point DLT homography (closed-form solve).

Contract: kernel(pts_1_tile, pred_h4p_tile) -> [B, 3, 3] float32, with
B = 524288 split across 8 NeuronCores (batch-parallel, no communication).

Math (per batch element, points p=0..3 with src (x_p,y_p), dst (X_p,Y_p)):
the DLT system rows are
    x h0 + y h1 + h2 = X (1 + x h6 + y h7)
    x h3 + y h4 + h5 = Y (1 + x h6 + y h7)
Eliminating (h0,h1,h2) from the four X-equations via the left null vector n
of M = [(x_p, y_p, 1)] gives one linear equation in (h6,h7); same for the
Y-equations. Solve the 2x2, back out the rest in closed form.

Layout: each core's 65536 elements sit at [128 partitions, 512 free]; every
per-element scalar is a [128, 512] "plane". Planes live at fixed offsets in
slabs so related planes are contiguous and most steps fuse into multi-plane
single instructions (positive-step / broadcast APs only — DVE runs those at
full rate). ScalarE does the interleave<->planar shuffles (with dtype
casts), VectorE + GPSIMD split the elementwise math, greedy-balanced.
Compute planes are fp16 (DVE 2x mode) or fp32; reciprocals and the 2x2
determinant stay fp32 either way.
"""
import sys

for _p in ("/opt/trn_rl_repo", "/root/.axon_site/_ro/trn_rl_repo"):
    if _p not in sys.path:
        sys.path.append(_p)

import numpy as np

import concourse.bass as bass
import concourse.mybir as mybir
from concourse import bacc
from concourse.tile import TileContext
from concourse.bass_utils import run_bass_kernel_spmd

N_CORES = 8
B_TOTAL = 524288
PER_CORE = B_TOTAL // N_CORES  # 65536
PARTS = 128
F = PER_CORE // PARTS  # 512
FP32 = mybir.dt.float32
FP16 = mybir.dt.float16

ADD = mybir.AluOpType.add
SUB = mybir.AluOpType.subtract
MUL = mybir.AluOpType.mult


class _Slab:
    """Bump allocator with explicit free, in F-plane units, first-fit."""

    def __init__(self, nplanes):
        self.free = [(0, nplanes)]
        self.nplanes = nplanes

    def alloc(self, n):
        for idx, (off, ln) in enumerate(self.free):
            if ln >= n:
                if ln == n:
                    self.free.pop(idx)
                else:
                    self.free[idx] = (off + n, ln - n)
                return off
        raise RuntimeError(f"slab OOM: need {n}, free={self.free}")

    def release(self, off, n):
        self.free.append((off, n))
        self.free.sort()
        merged = []
        for o, ln in self.free:
            if merged and merged[-1][0] + merged[-1][1] == o:
                merged[-1] = (merged[-1][0], merged[-1][1] + ln)
            else:
                merged.append([o, ln])
        self.free = [tuple(m) if isinstance(m, list) else m for m in merged]


class _Bal:
    """Greedy VectorE/GPSIMD balance by estimated op time (ns)."""

    def __init__(self, nc, fp16):
        self.nc = nc
        self.fp16 = fp16
        self.t_v = 0.0
        self.t_g = 0.0

    def cv(self, fd, bcast):
        acc = 2.0 if (self.fp16 and not bcast) else 1.0
        return (fd / acc + 151.0) / 0.96

    def cg(self, fd):
        return fd * 2.1 / 1.2 + 350.0

    def pick(self, fd, pin, bcast=False):
        cv, cg = self.cv(fd, bcast), self.cg(fd)
        if pin is None:
            eng = "v" if self.t_v + cv <= self.t_g + cg else "g"
        else:
            eng = pin
        if eng == "v":
            self.t_v += cv
            return self.nc.vector
        self.t_g += cg
        return self.nc.gpsimd


def _fd(ap):
    n = 1
    for d in ap.shape[1:]:
        n *= d
    return n


def _build(nchunk=1, fp16=False):
    fc = F // nchunk
    elems = PARTS * fc
    PDT = FP16 if fp16 else FP32

    nc = bacc.Bacc(None, target_bir_lowering=False, debug=True)
    pts = nc.dram_tensor("pts", [PER_CORE, 8], FP32, kind="ExternalInput")
    prd = nc.dram_tensor("prd", [PER_CORE, 8], FP32, kind="ExternalInput")
    out = nc.dram_tensor("out", [PER_CORE, 9], FP32, kind="ExternalOutput")

    # fp32 slab: DMA staging, output staging, recip/det planes
    N32 = 26
    # compute-plane slab (PDT dtype)
    NP = 58

    with TileContext(nc) as tc:
        nb = 1 if nchunk == 1 else 2
        with tc.tile_pool(name="s32", bufs=nb) as pool32, tc.tile_pool(
            name="sp", bufs=nb
        ) as poolp:
            for c in range(nchunk):
                slab32 = pool32.tile([PARTS, N32 * fc], FP32, tag="slab32")
                slabp = poolp.tile([PARTS, NP * fc], PDT, tag="slabp")
                sa32 = _Slab(N32)
                sa = _Slab(NP)
                bal = _Bal(nc, fp16)

                def R32(off, n):
                    return slab32[:, off * fc : (off + n) * fc]

                def R(off, n):
                    return slabp[:, off * fc : (off + n) * fc]

                def V(off, n):
                    return R(off, n).rearrange("p (c f) -> p c f", f=fc)

                def PL(off):
                    return R(off, 1)

                def BC(off, k):
                    return PL(off).unsqueeze(1).broadcast_to((PARTS, k, fc))

                def tt(o, a, b, op, pin=None, bcast=False):
                    eng = bal.pick(_fd(o), pin, bcast)
                    eng.tensor_tensor(out=o, in0=a, in1=b, op=op)

                def stt(o, in0, scalar, in1, op0, op1):
                    bal.t_v += bal.cv(_fd(o), False)
                    nc.vector.scalar_tensor_tensor(
                        out=o, in0=in0, scalar=scalar, in1=in1, op0=op0, op1=op1
                    )

                def scp(o, i):
                    nc.scalar.copy(out=o, in_=i)

                lo = c * elems
                hi = lo + elems

                vt = sa32.alloc(8)
                pt = sa32.alloc(8)
                ut = sa32.alloc(8)
                nc.sync.dma_start(
                    out=R32(vt, 8),
                    in_=pts[lo:hi, :].rearrange("(p f) c -> p (f c)", p=PARTS),
                )
                nc.sync.dma_start(
                    out=R32(pt, 8),
                    in_=prd[lo:hi, :].rearrange("(p f) c -> p (f c)", p=PARTS),
                )

                # interleaved u = v + pred (fp32, contiguous, split across V/G)
                tt(R32(ut, 8)[:, : 5 * fc], R32(vt, 8)[:, : 5 * fc],
                   R32(pt, 8)[:, : 5 * fc], ADD, pin="v")
                tt(R32(ut, 8)[:, 5 * fc :], R32(vt, 8)[:, 5 * fc :],
                   R32(pt, 8)[:, 5 * fc :], ADD, pin="g")

                # deinterleave (+ cast): comp (0,2,4,6,1,3,5,7) -> planar
                xv = sa.alloc(8)  # [x0,x1,x2,x3,y0,y1,y2,y3]
                uu = sa.alloc(8)  # [X0,X1,X2,X3,Y0,Y1,Y2,Y3]

                def deint(dst, src):
                    i = R32(src, 8).rearrange("p (f c g) -> p g c f", c=4, g=2)
                    o = R(dst, 8).rearrange("p (g c f) -> p g c f", c=4, g=2)
                    scp(o, i)

                deint(xv, vt)
                deint(uu, ut)
                sa32.release(vt, 8)
                sa32.release(pt, 8)
                sa32.release(ut, 8)
                ot = sa32.alloc(9)
                # OT is element-interleaved (f*9 + c): out-DMA is contiguous
                ov = R32(ot, 9).rearrange("p (f c) -> p c f", c=9)
                nc.vector.memset(ov[:, 8, :], 1.0)

                # diffs: D = [dx1,dx2,dx3,dy1,dy2,dy3]
                dd = sa.alloc(6)
                xv4 = R(xv, 8).rearrange("p (g c f) -> p g c f", g=2, c=4)
                in1 = (
                    V(xv, 8)[:, 0:5:4, :]
                    .unsqueeze(2)
                    .broadcast_to((PARTS, 2, 3, fc))
                )
                o = R(dd, 6).rearrange("p (g c f) -> p g c f", g=2, c=3)
                tt(o, xv4[:, :, 1:4, :], in1, SUB, pin="v", bcast=True)
                DX1, DX2, DX3, DY1, DY2, DY3 = range(dd, dd + 6)

                # n: n1=dx2dy3-dx3dy2, n2=dx3dy1-dx1dy3, n3=dx1dy2-dx2dy1
                pa = sa.alloc(3)
                pb = sa.alloc(3)
                for k, (a, b) in enumerate(((DX2, DY3), (DX3, DY1), (DX1, DY2))):
                    tt(PL(pa + k), PL(a), PL(b), MUL)
                for k, (a, b) in enumerate(((DX3, DY2), (DX1, DY3), (DX2, DY1))):
                    tt(PL(pb + k), PL(a), PL(b), MUL)
                ns32 = sa32.alloc(3)  # fp32 [n1,n2,n3] (n3 feeds recip)
                tt(R32(ns32, 3), R(pa, 3), R(pb, 3), SUB)
                ns = sa.alloc(4)  # PDT [n0,n1,n2,n3]
                scp(R(ns + 1, 3), R32(ns32, 3))
                t0 = sa.alloc(1)
                tt(PL(t0), PL(ns + 1), PL(ns + 2), ADD)
                stt(PL(ns), PL(t0), -1.0, PL(ns + 3), MUL, SUB)  # n0=-(n1+n2)-n3
                sa.release(pa, 3)
                sa.release(pb, 3)
                sa.release(t0, 1)

                # dots, grouped by point p: ZW[3p..] = (z_p, z_p x_p, z_p y_p)
                zx = sa.alloc(12)
                zy = sa.alloc(12)
                for zz, w in ((zx, 0), (zy, 4)):
                    tt(V(zz, 12)[:, 0:12:3, :], V(ns, 4), V(uu, 8)[:, w : w + 4, :],
                       MUL)
                    tt(V(zz, 12)[:, 1:12:3, :], V(zz, 12)[:, 0:12:3, :],
                       V(xv, 8)[:, 0:4, :], MUL)
                    tt(V(zz, 12)[:, 2:12:3, :], V(zz, 12)[:, 0:12:3, :],
                       V(xv, 8)[:, 4:8, :], MUL)
                tx = sa.alloc(6)
                tt(R(tx, 6), R(zx, 6), R(zx + 6, 6), ADD)
                sa.release(zx, 12)
                ty = sa.alloc(6)
                tt(R(ty, 6), R(zy, 6), R(zy + 6, 6), ADD)
                sa.release(zy, 12)
                ss = sa.alloc(6)  # [aX,bX,cX,aY,bY,cY]
                tt(R(ss, 3), R(tx, 3), R(tx + 3, 3), ADD)
                tt(R(ss + 3, 3), R(ty, 3), R(ty + 3, 3), ADD)
                sa.release(tx, 6)
                sa.release(ty, 6)

                # 2x2: det = bXcY-bYcX, h6n = cXaY-cYaX, h7n = bYaX-bXaY
                AX, BX, CX, AY, BY, CY = range(ss, ss + 6)
                pc = sa.alloc(3)
                pd = sa.alloc(3)
                for k, (a, b) in enumerate(((BX, CY), (CX, AY), (BY, AX))):
                    tt(PL(pc + k), PL(a), PL(b), MUL)
                for k, (a, b) in enumerate(((BY, CX), (CY, AX), (BX, AY))):
                    tt(PL(pd + k), PL(a), PL(b), MUL)
                dt32 = sa32.alloc(3)  # fp32 [det, h6n, h7n]
                tt(R32(dt32, 3), R(pc, 3), R(pd, 3), SUB)
                sa.release(pc, 3)
                sa.release(pd, 3)
                sa.release(ss, 6)

                rc32 = sa32.alloc(2)  # recip out + scratch
                nc.vector.reciprocal_approx_accurate(
                    out=R32(rc32, 1), in_=R32(dt32, 1), scratch=R32(rc32 + 1, 1)
                )
                bal.t_v += 2 * (fc + 151.0) / 0.96
                h67 = sa.alloc(2)
                # (h6,h7) = (h6n,h7n) * rdet ; mixed fp32 ins -> PDT out
                rdetb = (
                    R32(rc32, 1).unsqueeze(1).broadcast_to((PARTS, 2, fc))
                )
                tt(V(h67, 2), R32(dt32 + 1, 2).rearrange("p (c f) -> p c f", f=fc),
                   rdetb, MUL, pin="v", bcast=True)
                scp(ov[:, 6:8, :], V(h67, 2))
                sa32.release(dt32, 3)

                # XW_p = X_p (1 + x_p h6 + y_p h7), p=0..2; same for YW
                m1 = sa.alloc(3)
                m2 = sa.alloc(3)
                sp = sa.alloc(3)
                xw = sa.alloc(6)  # [XW0,XW1,XW2,YW0,YW1,YW2]
                tt(V(m1, 3), V(xv, 8)[:, 0:3, :], BC(h67, 3), MUL, pin="v",
                   bcast=True)
                tt(V(m2, 3), V(xv, 8)[:, 4:7, :], BC(h67 + 1, 3), MUL, pin="v",
                   bcast=True)
                tt(R(sp, 3), R(m1, 3), R(m2, 3), ADD)
                stt(V(xw, 6)[:, 0:3, :], V(sp, 3), 1.0, V(uu, 8)[:, 0:3, :],
                    ADD, MUL)
                stt(V(xw, 6)[:, 3:6, :], V(sp, 3), 1.0, V(uu, 8)[:, 4:7, :],
                    ADD, MUL)
                sa.release(m1, 3)
                sa.release(m2, 3)
                sa.release(sp, 3)
                sa.release(h67, 2)
                sa.release(uu, 8)

                # PQ = (XW1-XW0, XW2-XW0, YW1-YW0, YW2-YW0)
                pq = sa.alloc(4)
                xwv = R(xw, 6).rearrange("p (a b f) -> p a b f", a=2, b=3)
                tt(
                    R(pq, 4).rearrange("p (a b f) -> p a b f", a=2, b=2),
                    xwv[:, :, 1:3, :],
                    xwv[:, :, 0, :].unsqueeze(2).broadcast_to((PARTS, 2, 2, fc)),
                    SUB,
                    pin="v",
                    bcast=True,
                )

                # rD = 1 / n3  (fp32), then cast to PDT for the multiplies
                nc.vector.reciprocal_approx_accurate(
                    out=R32(rc32, 1), in_=R32(ns32 + 2, 1), scratch=R32(rc32 + 1, 1)
                )
                bal.t_v += 2 * (fc + 151.0) / 0.96
                rd = sa.alloc(1)
                scp(PL(rd), R32(rc32, 1))
                sa32.release(ns32, 3)
                sa.release(ns, 4)

                # pE = (P1 dy2, Q1 dy2, dx1 P2, dx1 Q2)
                # pF = (P2 dy1, Q2 dy1, dx2 P1, dx2 Q1)
                pe = sa.alloc(4)
                pf = sa.alloc(4)
                pqv = V(pq, 4)
                tt(V(pe, 4)[:, 0:2, :], pqv[:, 0:3:2, :], BC(DY2, 2), MUL,
                   pin="v", bcast=True)
                tt(V(pe, 4)[:, 2:4, :], pqv[:, 1:4:2, :], BC(DX1, 2), MUL,
                   pin="v", bcast=True)
                tt(V(pf, 4)[:, 0:2, :], pqv[:, 1:4:2, :], BC(DY1, 2), MUL,
                   pin="v", bcast=True)
                tt(V(pf, 4)[:, 2:4, :], pqv[:, 0:3:2, :], BC(DX2, 2), MUL,
                   pin="v", bcast=True)
                hn = sa.alloc(4)  # [h0n, h3n, h1n, h4n]
                tt(R(hn, 4), R(pe, 4), R(pf, 4), SUB)
                hg = sa.alloc(4)  # [h0, h3, h1, h4]
                tt(V(hg, 4), V(hn, 4), BC(rd, 4), MUL, pin="v", bcast=True)
                sa.release(pe, 4)
                sa.release(pf, 4)
                sa.release(hn, 4)
                sa.release(pq, 4)
                sa.release(rd, 1)
                sa32.release(rc32, 2)

                scp(ov[:, 0:4:3, :], V(hg, 2))       # h0, h3
                scp(ov[:, 1:5:3, :], V(hg + 2, 2))   # h1, h4

                # h2 = XW0 - x0 h0 - y0 h1 ; h5 = YW0 - x0 h3 - y0 h4
                ee = sa.alloc(4)  # (x0 h0, y0 h1, x0 h3, y0 h4)
                in0 = (
                    V(xv, 8)[:, 0:5:4, :]
                    .unsqueeze(1)
                    .broadcast_to((PARTS, 2, 2, fc))
                )
                in1 = R(hg, 4).rearrange("p (a b f) -> p b a f", a=2, b=2)
                tt(R(ee, 4).rearrange("p (g h f) -> p g h f", g=2, h=2),
                   in0, in1, MUL, pin="v", bcast=True)
                s1 = sa.alloc(2)
                eev = V(ee, 4)
                tt(V(s1, 2), V(xw, 6)[:, 0:4:3, :], eev[:, 0:3:2, :], SUB, pin="v")
                h25 = sa.alloc(2)
                tt(V(h25, 2), V(s1, 2), eev[:, 1:4:2, :], SUB, pin="v")
                scp(ov[:, 2:6:3, :], V(h25, 2))
                sa.release(ee, 4)
                sa.release(s1, 2)
                sa.release(hg, 4)
                sa.release(xw, 6)
                sa.release(dd, 6)
                sa.release(xv, 8)
                sa.release(h25, 2)

                nc.sync.dma_start(
                    out=out[lo:hi, :].rearrange("(p f) c -> p (f c)", p=PARTS),
                    in_=R32(ot, 9),
                )
                sa32.release(ot, 9)
    nc.finalize()
    return nc


_NC_CACHE = {}


def _get_nc(nchunk=1, fp16=False):
    key = (nchunk, fp16)
    if key not in _NC_CACHE:
        _NC_CACHE[key] = _build(nchunk, fp16)
    return _NC_CACHE[key]


def kernel(pts_1_tile, pred_h4p_tile, _trace=False, _nchunk=1, _fp16=False):
    pts = np.ascontiguousarray(
        np.asarray(pts_1_tile, dtype=np.float32).reshape(B_TOTAL, 8)
    )
    prd = np.ascontiguousarray(
        np.asarray(pred_h4p_tile, dtype=np.float32).reshape(B_TOTAL, 8)
    )
    nc = _get_nc(_nchunk, _fp16)
    in_maps = [
        {
            "pts": pts[i * PER_CORE : (i + 1) * PER_CORE],
            "prd": prd[i * PER_CORE : (i + 1) * PER_CORE],
        }
        for i in range(N_CORES)
    ]
    res = run_bass_kernel_spmd(nc, in_maps, list(range(N_CORES)), trace=_trace)
    outs = np.concatenate([res.results[i]["out"] for i in range(N_CORES)], axis=0)
    H = outs.reshape(B_TOTAL, 3, 3).astype(np.float32)
    if _trace:
        return H, res
    return H


# revision 19
# speedup vs baseline: 1.5948x; 1.0288x over previous
"""Trainium2 Bass kernel: batched 4-point DLT homography (closed-form solve).

Contract: kernel(pts_1_tile, pred_h4p_tile) -> [B, 3, 3] float32, with
B = 524288 split across 8 NeuronCores (batch-parallel, no communication).

Math (per batch element, points p=0..3 with src (x_p,y_p), dst (X_p,Y_p)):
the DLT system rows are
    x h0 + y h1 + h2 = X (1 + x h6 + y h7)
    x h3 + y h4 + h5 = Y (1 + x h6 + y h7)
Eliminating (h0,h1,h2) from the four X-equations via the left null vector n
of M = [(x_p, y_p, 1)] gives one linear equation in (h6,h7); same for the
Y-equations. Solve the 2x2, back out the rest in closed form.

Layout: each core's 65536 elements sit at [128 partitions, 512 free]; every
per-element scalar is a [128, 512] "plane". Planes live at fixed offsets in
slabs so related planes are contiguous and most steps fuse into multi-plane
single instructions (positive-step / broadcast APs only — DVE runs those at
full rate). ScalarE does the interleave<->planar shuffles (with dtype
casts), VectorE + GPSIMD split the elementwise math, greedy-balanced.
Compute planes are fp16 (DVE 2x mode) or fp32; reciprocals and the 2x2
determinant stay fp32 either way.
"""
import sys

for _p in ("/opt/trn_rl_repo", "/root/.axon_site/_ro/trn_rl_repo"):
    if _p not in sys.path:
        sys.path.append(_p)

import numpy as np

import concourse.bass as bass
import concourse.mybir as mybir
from concourse import bacc
from concourse.tile import TileContext
from concourse.bass_utils import run_bass_kernel_spmd

N_CORES = 8
B_TOTAL = 524288
PER_CORE = B_TOTAL // N_CORES  # 65536
PARTS = 128
F = PER_CORE // PARTS  # 512
FP32 = mybir.dt.float32
FP16 = mybir.dt.float16

ADD = mybir.AluOpType.add
SUB = mybir.AluOpType.subtract
MUL = mybir.AluOpType.mult


class _Slab:
    """Bump allocator with explicit free, in F-plane units, first-fit."""

    def __init__(self, nplanes):
        self.free = [(0, nplanes)]
        self.nplanes = nplanes

    def alloc(self, n):
        for idx, (off, ln) in enumerate(self.free):
            if ln >= n:
                if ln == n:
                    self.free.pop(idx)
                else:
                    self.free[idx] = (off + n, ln - n)
                return off
        raise RuntimeError(f"slab OOM: need {n}, free={self.free}")

    def release(self, off, n):
        self.free.append((off, n))
        self.free.sort()
        merged = []
        for o, ln in self.free:
            if merged and merged[-1][0] + merged[-1][1] == o:
                merged[-1] = (merged[-1][0], merged[-1][1] + ln)
            else:
                merged.append([o, ln])
        self.free = [tuple(m) if isinstance(m, list) else m for m in merged]


class _Bal:
    """Greedy VectorE/GPSIMD balance by estimated op time (ns)."""

    def __init__(self, nc, fp16):
        self.nc = nc
        self.fp16 = fp16
        self.t_v = 0.0
        self.t_g = 0.0

    def cv(self, fd, bcast):
        acc = 2.0 if self.fp16 else 1.0
        return (fd / acc + 64.0) / 0.96

    def cg(self, fd):
        return fd * 2.05 / 1.2 + 350.0

    def pick(self, fd, pin, bcast=False):
        cv, cg = self.cv(fd, bcast), self.cg(fd)
        if pin is None:
            eng = "v" if self.t_v + cv <= self.t_g + cg else "g"
        else:
            eng = pin
        if eng == "v":
            self.t_v += cv
            return self.nc.vector
        self.t_g += cg
        return self.nc.gpsimd


def _fd(ap):
    n = 1
    for d in ap.shape[1:]:
        n *= d
    return n


OPLOG = {}


def _build(nchunk=1, fp16=False):
    OPLOG.clear()
    fc = F // nchunk
    elems = PARTS * fc
    PDT = FP16 if fp16 else FP32

    nc = bacc.Bacc(None, target_bir_lowering=False, debug=True)
    pts = nc.dram_tensor("pts", [PER_CORE, 8], FP32, kind="ExternalInput")
    prd = nc.dram_tensor("prd", [PER_CORE, 8], FP32, kind="ExternalInput")
    out = nc.dram_tensor("out", [PER_CORE, 9], FP32, kind="ExternalOutput")

    # fp32 slab: DMA staging, output staging, recip/det planes
    N32 = 26
    # compute-plane slab (PDT dtype)
    NP = 58

    with TileContext(nc) as tc:
        nb = 1 if nchunk == 1 else 2
        with tc.tile_pool(name="s32", bufs=nb) as pool32, tc.tile_pool(
            name="sp", bufs=nb
        ) as poolp:
            for c in range(nchunk):
                slab32 = pool32.tile([PARTS, N32 * fc], FP32, tag="slab32")
                slabp = poolp.tile([PARTS, NP * fc], PDT, tag="slabp")
                sa32 = _Slab(N32)
                sa = _Slab(NP)
                bal = _Bal(nc, fp16)

                def R32(off, n):
                    return slab32[:, off * fc : (off + n) * fc]

                def R(off, n):
                    return slabp[:, off * fc : (off + n) * fc]

                def V(off, n):
                    return R(off, n).rearrange("p (c f) -> p c f", f=fc)

                def PL(off):
                    return R(off, 1)

                def BC(off, k):
                    return PL(off).unsqueeze(1).broadcast_to((PARTS, k, fc))

                def tt(o, a, b, op, pin=None, bcast=False, desc=""):
                    eng = bal.pick(_fd(o), pin, bcast)
                    ins = eng.tensor_tensor(out=o, in0=a, in1=b, op=op)
                    OPLOG[ins.ins.name] = desc or "tt"

                def stt(o, in0, scalar, in1, op0, op1, desc="stt"):
                    bal.t_v += bal.cv(_fd(o), False)
                    ins = nc.vector.scalar_tensor_tensor(
                        out=o, in0=in0, scalar=scalar, in1=in1, op0=op0, op1=op1
                    )
                    OPLOG[ins.ins.name] = desc

                def scp(o, i, desc="scp"):
                    ins = nc.scalar.copy(out=o, in_=i)
                    OPLOG[ins.ins.name] = desc

                lo = c * elems
                hi = lo + elems

                vt = sa32.alloc(8)
                pt = sa32.alloc(8)
                ut = sa32.alloc(8)
                nc.sync.dma_start(
                    out=R32(vt, 8),
                    in_=pts[lo:hi, :].rearrange("(p f) c -> p (f c)", p=PARTS),
                )
                nc.sync.dma_start(
                    out=R32(pt, 8),
                    in_=prd[lo:hi, :].rearrange("(p f) c -> p (f c)", p=PARTS),
                )

                # interleaved u = v + pred (fp32, contiguous, split across V/G)
                tt(R32(ut, 8)[:, : 5 * fc], R32(vt, 8)[:, : 5 * fc],
                   R32(pt, 8)[:, : 5 * fc], ADD, pin="v", desc="uaddV")
                tt(R32(ut, 8)[:, 5 * fc :], R32(vt, 8)[:, 5 * fc :],
                   R32(pt, 8)[:, 5 * fc :], ADD, pin="g", desc="uaddG")

                # deinterleave (+ cast): comp (0,2,4,6,1,3,5,7) -> planar
                xv = sa.alloc(8)  # [x0,x1,x2,x3,y0,y1,y2,y3]
                uu = sa.alloc(8)  # [X0,X1,X2,X3,Y0,Y1,Y2,Y3]

                def deint(dst, src):
                    i = R32(src, 8).rearrange("p (f c g) -> p g c f", c=4, g=2)
                    o = R(dst, 8).rearrange("p (g c f) -> p g c f", c=4, g=2)
                    scp(o, i)

                deint(xv, vt)
                deint(uu, ut)
                sa32.release(vt, 8)
                sa32.release(pt, 8)
                sa32.release(ut, 8)
                ot = sa32.alloc(9)
                # OT is element-interleaved (f*9 + c): out-DMA is contiguous
                ov = R32(ot, 9).rearrange("p (f c) -> p c f", c=9)
                nc.vector.memset(ov[:, 8, :], 1.0)

                # diffs: D = [dx1,dx2,dx3,dy1,dy2,dy3]
                dd = sa.alloc(6)
                xv4 = R(xv, 8).rearrange("p (g c f) -> p g c f", g=2, c=4)
                in1 = (
                    V(xv, 8)[:, 0:5:4, :]
                    .unsqueeze(2)
                    .broadcast_to((PARTS, 2, 3, fc))
                )
                o = R(dd, 6).rearrange("p (g c f) -> p g c f", g=2, c=3)
                tt(o, xv4[:, :, 1:4, :], in1, SUB, pin="v", bcast=True, desc="diffs")
                DX1, DX2, DX3, DY1, DY2, DY3 = range(dd, dd + 6)

                # n: n1=dx2dy3-dx3dy2, n2=dx3dy1-dx1dy3, n3=dx1dy2-dx2dy1
                pa = sa.alloc(3)
                pb = sa.alloc(3)
                for k, (a, b) in enumerate(((DX2, DY3), (DX3, DY1), (DX1, DY2))):
                    tt(PL(pa + k), PL(a), PL(b), MUL, desc=f"pa{k}")
                for k, (a, b) in enumerate(((DX3, DY2), (DX1, DY3), (DX2, DY1))):
                    tt(PL(pb + k), PL(a), PL(b), MUL, desc=f"pb{k}")
                ns32 = sa32.alloc(3)  # fp32 [n1,n2,n3] (n3 feeds recip)
                tt(R32(ns32, 3), R(pa, 3), R(pb, 3), SUB, desc="ns32sub")
                ns = sa.alloc(4)  # PDT [n0,n1,n2,n3]
                scp(R(ns + 1, 3), R32(ns32, 3))
                t0 = sa.alloc(1)
                tt(PL(t0), PL(ns + 1), PL(ns + 2), ADD, desc="t0")
                stt(PL(ns), PL(t0), -1.0, PL(ns + 3), MUL, SUB)  # n0=-(n1+n2)-n3
                sa.release(pa, 3)
                sa.release(pb, 3)
                sa.release(t0, 1)

                # dots, grouped by point p: ZW[3p..] = (z_p, z_p x_p, z_p y_p)
                zx = sa.alloc(12)
                zy = sa.alloc(12)
                for zz, w in ((zx, 0), (zy, 4)):
                    tt(V(zz, 12)[:, 0:12:3, :], V(ns, 4), V(uu, 8)[:, w : w + 4, :],
                       MUL, desc=f"z{w}")
                    tt(V(zz, 12)[:, 1:12:3, :], V(zz, 12)[:, 0:12:3, :],
                       V(xv, 8)[:, 0:4, :], MUL, desc=f"q{w}")
                    tt(V(zz, 12)[:, 2:12:3, :], V(zz, 12)[:, 0:12:3, :],
                       V(xv, 8)[:, 4:8, :], MUL, desc=f"r{w}")
                tx = sa.alloc(6)
                tt(R(tx, 6), R(zx, 6), R(zx + 6, 6), ADD, desc="TX")
                sa.release(zx, 12)
                ty = sa.alloc(6)
                tt(R(ty, 6), R(zy, 6), R(zy + 6, 6), ADD, desc="TY")
                sa.release(zy, 12)
                ss = sa.alloc(6)  # [aX,bX,cX,aY,bY,cY]
                tt(R(ss, 3), R(tx, 3), R(tx + 3, 3), ADD, desc="ssX")
                tt(R(ss + 3, 3), R(ty, 3), R(ty + 3, 3), ADD, desc="ssY")
                sa.release(tx, 6)
                sa.release(ty, 6)

                # 2x2: det = bXcY-bYcX, h6n = cXaY-cYaX, h7n = bYaX-bXaY
                AX, BX, CX, AY, BY, CY = range(ss, ss + 6)
                pc = sa.alloc(3)
                pd = sa.alloc(3)
                for k, (a, b) in enumerate(((BX, CY), (CX, AY), (BY, AX))):
                    tt(PL(pc + k), PL(a), PL(b), MUL, desc=f"pc{k}")
                for k, (a, b) in enumerate(((BY, CX), (CY, AX), (BX, AY))):
                    tt(PL(pd + k), PL(a), PL(b), MUL, desc=f"pd{k}")
                dt32 = sa32.alloc(3)  # fp32 [det, h6n, h7n]
                tt(R32(dt32, 3), R(pc, 3), R(pd, 3), SUB, desc="dtsub")
                sa.release(pc, 3)
                sa.release(pd, 3)
                sa.release(ss, 6)

                rc32 = sa32.alloc(2)  # recip out + scratch
                nc.vector.reciprocal_approx_accurate(
                    out=R32(rc32, 1), in_=R32(dt32, 1), scratch=R32(rc32 + 1, 1)
                )
                bal.t_v += 2 * (fc + 151.0) / 0.96
                h67 = sa.alloc(2)
                # (h6,h7) = (h6n,h7n) * rdet ; mixed fp32 ins -> PDT out
                rdetb = (
                    R32(rc32, 1).unsqueeze(1).broadcast_to((PARTS, 2, fc))
                )
                tt(V(h67, 2), R32(dt32 + 1, 2).rearrange("p (c f) -> p c f", f=fc),
                   rdetb, MUL, pin="v", bcast=True, desc="h67")
                scp(ov[:, 6:8, :], V(h67, 2))
                sa32.release(dt32, 3)

                # XW_p = X_p (1 + x_p h6 + y_p h7), p=0..2; same for YW
                m1 = sa.alloc(3)
                m2 = sa.alloc(3)
                sp = sa.alloc(3)
                xw = sa.alloc(6)  # [XW0,XW1,XW2,YW0,YW1,YW2]
                tt(V(m1, 3), V(xv, 8)[:, 0:3, :], BC(h67, 3), MUL, pin="v",
                   bcast=True, desc="m1")
                tt(V(m2, 3), V(xv, 8)[:, 4:7, :], BC(h67 + 1, 3), MUL, pin="v",
                   bcast=True, desc="m2")
                tt(R(sp, 3), R(m1, 3), R(m2, 3), ADD, desc="sp")
                stt(V(xw, 6)[:, 0:3, :], V(sp, 3), 1.0, V(uu, 8)[:, 0:3, :],
                    ADD, MUL)
                stt(V(xw, 6)[:, 3:6, :], V(sp, 3), 1.0, V(uu, 8)[:, 4:7, :],
                    ADD, MUL)
                sa.release(m1, 3)
                sa.release(m2, 3)
                sa.release(sp, 3)
                sa.release(h67, 2)
                sa.release(uu, 8)

                # PQ = (XW1-XW0, XW2-XW0, YW1-YW0, YW2-YW0)
                pq = sa.alloc(4)
                xwv = R(xw, 6).rearrange("p (a b f) -> p a b f", a=2, b=3)
                tt(
                    R(pq, 4).rearrange("p (a b f) -> p a b f", a=2, b=2),
                    xwv[:, :, 1:3, :],
                    xwv[:, :, 0, :].unsqueeze(2).broadcast_to((PARTS, 2, 2, fc)),
                    SUB,
                    pin="v",
                    bcast=True,
                    desc="PQ",
                )

                # rD = 1 / n3  (fp32), then cast to PDT for the multiplies
                nc.vector.reciprocal_approx_accurate(
                    out=R32(rc32, 1), in_=R32(ns32 + 2, 1), scratch=R32(rc32 + 1, 1)
                )
                bal.t_v += 2 * (fc + 151.0) / 0.96
                rd = sa.alloc(1)
                scp(PL(rd), R32(rc32, 1))
                sa32.release(ns32, 3)
                sa.release(ns, 4)

                # pE = (P1 dy2, Q1 dy2, dx1 P2, dx1 Q2)
                # pF = (P2 dy1, Q2 dy1, dx2 P1, dx2 Q1)
                pe = sa.alloc(4)
                pf = sa.alloc(4)
                pqv = V(pq, 4)
                tt(V(pe, 4)[:, 0:2, :], pqv[:, 0:3:2, :], BC(DY2, 2), MUL,
                   pin="v", bcast=True, desc="pe01")
                tt(V(pe, 4)[:, 2:4, :], pqv[:, 1:4:2, :], BC(DX1, 2), MUL,
                   pin="v", bcast=True, desc="pe23")
                tt(V(pf, 4)[:, 0:2, :], pqv[:, 1:4:2, :], BC(DY1, 2), MUL,
                   pin="v", bcast=True, desc="pf01")
                tt(V(pf, 4)[:, 2:4, :], pqv[:, 0:3:2, :], BC(DX2, 2), MUL,
                   pin="v", bcast=True, desc="pf23")
                hn = sa.alloc(4)  # [h0n, h3n, h1n, h4n]
                tt(R(hn, 4), R(pe, 4), R(pf, 4), SUB, desc="hn")
                hg = sa.alloc(4)  # [h0, h3, h1, h4]
                tt(V(hg, 4), V(hn, 4), BC(rd, 4), MUL, pin="v", bcast=True, desc="hg")
                sa.release(pe, 4)
                sa.release(pf, 4)
                sa.release(hn, 4)
                sa.release(pq, 4)
                sa.release(rd, 1)
                sa32.release(rc32, 2)

                scp(ov[:, 0:4:3, :], V(hg, 2))       # h0, h3
                scp(ov[:, 1:5:3, :], V(hg + 2, 2))   # h1, h4

                # h2 = XW0 - x0 h0 - y0 h1 ; h5 = YW0 - x0 h3 - y0 h4
                ee = sa.alloc(4)  # (x0 h0, y0 h1, x0 h3, y0 h4)
                in0 = (
                    V(xv, 8)[:, 0:5:4, :]
                    .unsqueeze(1)
                    .broadcast_to((PARTS, 2, 2, fc))
                )
                in1 = R(hg, 4).rearrange("p (a b f) -> p b a f", a=2, b=2)
                tt(R(ee, 4).rearrange("p (g h f) -> p g h f", g=2, h=2),
                   in0, in1, MUL, pin="v", bcast=True, desc="ee")
                s1 = sa.alloc(2)
                eev = V(ee, 4)
                tt(V(s1, 2), V(xw, 6)[:, 0:4:3, :], eev[:, 0:3:2, :], SUB, pin="v", desc="s1")
                h25 = sa.alloc(2)
                tt(V(h25, 2), V(s1, 2), eev[:, 1:4:2, :], SUB, pin="v", desc="h25")
                scp(ov[:, 2:6:3, :], V(h25, 2))
                sa.release(ee, 4)
                sa.release(s1, 2)
                sa.release(hg, 4)
                sa.release(xw, 6)
                sa.release(dd, 6)
                sa.release(xv, 8)
                sa.release(h25, 2)

                nc.sync.dma_start(
                    out=out[lo:hi, :].rearrange("(p f) c -> p (f c)", p=PARTS),
                    in_=R32(ot, 9),
                )
                sa32.release(ot, 9)
    nc.finalize()
    return nc


_NC_CACHE = {}


def _get_nc(nchunk=1, fp16=False):
    key = (nchunk, fp16)
    if key not in _NC_CACHE:
        _NC_CACHE[key] = _build(nchunk, fp16)
    return _NC_CACHE[key]


def kernel(pts_1_tile, pred_h4p_tile, _trace=False, _nchunk=1, _fp16=False):
    pts = np.ascontiguousarray(
        np.asarray(pts_1_tile, dtype=np.float32).reshape(B_TOTAL, 8)
    )
    prd = np.ascontiguousarray(
        np.asarray(pred_h4p_tile, dtype=np.float32).reshape(B_TOTAL, 8)
    )
    nc = _get_nc(_nchunk, _fp16)
    in_maps = [
        {
            "pts": pts[i * PER_CORE : (i + 1) * PER_CORE],
            "prd": prd[i * PER_CORE : (i + 1) * PER_CORE],
        }
        for i in range(N_CORES)
    ]
    res = run_bass_kernel_spmd(nc, in_maps, list(range(N_CORES)), trace=_trace)
    outs = np.concatenate([res.results[i]["out"] for i in range(N_CORES)], axis=0)
    H = outs.reshape(B_TOTAL, 3, 3).astype(np.float32)
    if _trace:
        return H, res
    return H
